# revision 16
# baseline (speedup 1.0000x reference)
"""Trainium2 Bass kernel for nn_HamiltonianBlock.

Reference computes (q, p, H) where q,p pass through unchanged and H is a
scalar: sums of linear/quadratic terms of per-batch token sums, low-rank
cubic terms, and a token MLP, all reduced over batch+sequence.

Strategy (8 cores, data-parallel over sequence):
  - shard tokens: core c gets q[:, c*1024:(c+1)*1024, :] -> [4096, 256]
  - on device per core:
      * PE transposes fp32 token tiles -> xT (feature-major), DVE
        tensor_tensor_reduce evacuates PSUM -> bf16 xT while emitting
        exact fp32 per-feature token sums (q_sum / p_sum partials)
      * cubic projections p1,p2,p3,q3 via bf16 matmuls (token-major out),
        DVE elementwise + reduce for ppp / qpq partials
      * token MLP in bf16 on PE; gelu (erf) + bias on ACT; final layer
        folded into a per-feature token sum (accum_out) dotted with w3
  - host: gather tiny per-core partials, finish the O(B*D) algebra in
    float64, return (q, p, H).

Precision: H is dominated (~99.9%) by the quadratic terms, which are
computed from exact fp32 token sums; the bf16 MLP/cubic parts contribute
~1.5e-3 of |H| so their bf16 error is ~1e-6 relative on H.
"""

import os

os.environ.setdefault("BASS_NEVER_TRACE", "1")

import numpy as np
import ml_dtypes

from contextlib import ExitStack

import concourse.bass as bass
import concourse.tile as tile
from concourse import bacc, mybir
from concourse.bass_utils import run_bass_kernel_spmd
from concourse.masks import make_identity

F32 = mybir.dt.float32
BF16 = mybir.dt.bfloat16

B, S, D = 4, 8192, 256
HID = 512
R = 32
NCORES = 8
SS = S // NCORES            # 1024 tokens of each batch per core
T = B * SS                  # 4096 tokens per core
P = 128                     # partition dim
NCHUNK = T // 512           # 8 chunks of 512 tokens
NBLK = 4                    # 128-token blocks per chunk

_CACHED = {}

DO_CUBIC = os.environ.get("K_NO_CUBIC", "") == ""
DO_MLP = os.environ.get("K_NO_MLP", "") == ""
DO_TP = os.environ.get("K_NO_TP", "") == ""


def _build_program():
    nc = bacc.Bacc(
        "TRN2",
        target_bir_lowering=False,
        debug=False,
        num_devices=NCORES,
    )

    qs = nc.dram_tensor("qs", [T, D], F32, kind="ExternalInput").ap()
    ps = nc.dram_tensor("ps", [T, D], F32, kind="ExternalInput").ap()
    wp = nc.dram_tensor("wp", [D, 3 * R], BF16, kind="ExternalInput").ap()
    wq = nc.dram_tensor("wq", [D, R], BF16, kind="ExternalInput").ap()
    w1 = nc.dram_tensor("w1", [2 * D, 2 * HID], BF16, kind="ExternalInput").ap()
    w2 = nc.dram_tensor("w2", [2 * HID, HID], BF16, kind="ExternalInput").ap()
    b1c = nc.dram_tensor("b1c", [P, 8], F32, kind="ExternalInput").ap()
    b2c = nc.dram_tensor("b2c", [P, 4], F32, kind="ExternalInput").ap()
    w3c = nc.dram_tensor("w3c", [P, 4], F32, kind="ExternalInput").ap()

    sums = nc.dram_tensor("sums", [P, 16], F32, kind="ExternalOutput").ap()
    cub = nc.dram_tensor("cub", [P, 2], F32, kind="ExternalOutput").ap()
    mlp = nc.dram_tensor("mlp", [P, 1], F32, kind="ExternalOutput").ap()

    MULT = mybir.AluOpType.mult
    ADD = mybir.AluOpType.add
    AX = mybir.AxisListType.X
    GELU = mybir.ActivationFunctionType.Gelu

    with tile.TileContext(nc) as tc, ExitStack() as ctx:
        consts = ctx.enter_context(tc.tile_pool(name="consts", bufs=1))
        wpool = ctx.enter_context(tc.tile_pool(name="wpool", bufs=1))
        accp = ctx.enter_context(tc.tile_pool(name="accp", bufs=1))
        xtp_pool = ctx.enter_context(tc.tile_pool(name="xt", bufs=1))
        xin = ctx.enter_context(tc.tile_pool(name="xin", bufs=3))
        scr = ctx.enter_context(tc.tile_pool(name="scr", bufs=3))

        if DO_MLP:
            h1p = ctx.enter_context(tc.tile_pool(name="h1p", bufs=12))
            mm1_ps = ctx.enter_context(
                tc.tile_pool(name="mm1_ps", bufs=2, space=bass.MemorySpace.PSUM)
            )
            mm2_ps = ctx.enter_context(
                tc.tile_pool(name="mm2_ps", bufs=2, space=bass.MemorySpace.PSUM)
            )
        if DO_TP:
            tp_ps = ctx.enter_context(
                tc.tile_pool(name="tp_ps", bufs=3, space=bass.MemorySpace.PSUM)
            )
        if DO_CUBIC:
            cub_ps = ctx.enter_context(
                tc.tile_pool(name="cub_ps", bufs=1, space=bass.MemorySpace.PSUM)
            )

        # ---- constants & weights -------------------------------------
        identity = consts.tile([P, P], F32)
        make_identity(nc, identity[:])

        w1sb = wpool.tile([P, 4 * 2 * HID], BF16)   # [128, 4096]
        for k in range(4):
            nc.sync.dma_start(
                out=w1sb[:, k * 1024 : (k + 1) * 1024],
                in_=w1[k * P : (k + 1) * P, :],
            )
        w2sb = wpool.tile([P, 8 * HID], BF16)       # [128, 4096]
        for k in range(8):
            nc.sync.dma_start(
                out=w2sb[:, k * HID : (k + 1) * HID],
                in_=w2[k * P : (k + 1) * P, :],
            )
        wpsb = wpool.tile([P, 2 * 3 * R], BF16)     # [128, 192]
        wqsb = wpool.tile([P, 2 * R], BF16)         # [128, 64]
        for h in range(2):
            nc.sync.dma_start(
                out=wpsb[:, h * 96 : (h + 1) * 96], in_=wp[h * P : (h + 1) * P, :]
            )
            nc.sync.dma_start(
                out=wqsb[:, h * R : (h + 1) * R], in_=wq[h * P : (h + 1) * P, :]
            )
        b1sb = wpool.tile([P, 8], F32)
        nc.sync.dma_start(out=b1sb[:], in_=b1c[:])
        b2sb = wpool.tile([P, 4], F32)
        nc.sync.dma_start(out=b2sb[:], in_=b2c[:])
        w3sb = wpool.tile([P, 4], F32)
        nc.sync.dma_start(out=w3sb[:], in_=w3c[:])

        # ---- accumulators --------------------------------------------
        if DO_TP:
            qsums = accp.tile([P, 4 * 32], F32)  # (t*2+half)*32 + chunk*4+blk
            sums_sb = accp.tile([P, 16], F32)
        if DO_CUBIC:
            pppsl = accp.tile([P, 32], F32)
            qpqsl = accp.tile([P, 32], F32)
            cub_sb = accp.tile([P, 2], F32)
        if DO_MLP:
            h2sl = accp.tile([P, 32], F32)       # m2*8 + chunk
            h2sum = accp.tile([P, 4], F32)
            mlp_sb = accp.tile([P, 1], F32)

        # xT buffers: [128, half, T] feature-major bf16
        xtq = xtp_pool.tile([P, 2, T], BF16)
        xtp = xtp_pool.tile([P, 2, T], BF16)

        for c in range(NCHUNK):
            tok0 = c * 512
            qin = xin.tile([P, NBLK, D], F32, tag="qin")
            nc.sync.dma_start(
                out=qin[:],
                in_=qs[tok0 : tok0 + 512, :].rearrange("(n p) d -> p n d", p=P),
            )
            pin = xin.tile([P, NBLK, D], F32, tag="pin")
            nc.sync.dma_start(
                out=pin[:],
                in_=ps[tok0 : tok0 + 512, :].rearrange("(n p) d -> p n d", p=P),
            )

            # transposes + token-sum partials + bf16 evacuation
            for t, (xi, xt) in enumerate(((qin, xtq), (pin, xtp))) if DO_TP else ():
                for h in range(2):
                    for j in range(NBLK):
                        psb = tp_ps.tile([P, P], F32)
                        nc.tensor.transpose(
                            psb[:], xi[:, j, h * P : (h + 1) * P], identity[:]
                        )
                        slot = (t * 2 + h) * 32 + c * 4 + j
                        nc.vector.tensor_copy(
                            xt[:, h, tok0 + j * P : tok0 + (j + 1) * P], psb[:]
                        )
                        nc.vector.tensor_reduce(
                            qsums[:, slot : slot + 1], psb[:], AX, ADD
                        )

            # cubic terms, token-major [128 tok, R]
            for j in range(NBLK) if DO_CUBIC else ():
                cps = cub_ps.tile([P, P], F32)
                for h in range(2):
                    nc.tensor.matmul(
                        cps[:, 0:96],
                        lhsT=xtp[:, h, tok0 + j * P : tok0 + (j + 1) * P],
                        rhs=wpsb[:, h * 96 : (h + 1) * 96],
                        start=(h == 0),
                        stop=(h == 1),
                        skip_group_check=True,
                    )
                for h in range(2):
                    nc.tensor.matmul(
                        cps[:, 96:128],
                        lhsT=xtq[:, h, tok0 + j * P : tok0 + (j + 1) * P],
                        rhs=wqsb[:, h * R : (h + 1) * R],
                        start=(h == 0),
                        stop=(h == 1),
                        skip_group_check=True,
                    )
                s12 = scr.tile([P, 2 * R], F32, tag="s12")
                nc.vector.tensor_copy(s12[:], cps[:, 0:64])
                p12 = scr.tile([P, R], F32, tag="p12")
                nc.vector.tensor_mul(p12[:], s12[:, 0:32], s12[:, 32:64])
                d1 = scr.tile([P, R], F32, tag="ttrd")
                nc.vector.tensor_mul(d1[:], p12[:], cps[:, 64:96])
                nc.vector.tensor_reduce(
                    pppsl[:, c * 4 + j : c * 4 + j + 1], d1[:], AX, ADD
                )
                d2 = scr.tile([P, R], F32, tag="ttrd")
                nc.vector.tensor_mul(d2[:], p12[:], cps[:, 96:128])
                nc.vector.tensor_reduce(
                    qpqsl[:, c * 4 + j : c * 4 + j + 1], d2[:], AX, ADD
                )

            # MLP layer 1: h1T[m] = gelu(w1[:,m].T @ xT + b1[m])
            h1list = []
            for m in range(8) if DO_MLP else ():
                ps1 = mm1_ps.tile([P, 512], F32)
                for k in range(4):
                    xt = xtq if k < 2 else xtp
                    h = k % 2
                    nc.tensor.matmul(
                        ps1[:],
                        lhsT=w1sb[:, k * 1024 + m * P : k * 1024 + (m + 1) * P],
                        rhs=xt[:, h, tok0 : tok0 + 512],
                        start=(k == 0),
                        stop=(k == 3),
                    )
                h1t = h1p.tile([P, 512], BF16)
                nc.scalar.activation(
                    h1t[:], ps1[:], GELU, bias=b1sb[:, m : m + 1], scale=1.0
                )
                h1list.append(h1t)

            # MLP layer 2 + fused token reduction
            for m2 in range(4) if DO_MLP else ():
                ps2 = mm2_ps.tile([P, 512], F32)
                for kk in range(8):
                    nc.tensor.matmul(
                        ps2[:],
                        lhsT=w2sb[:, kk * HID + m2 * P : kk * HID + (m2 + 1) * P],
                        rhs=h1list[kk][:],
                        start=(kk == 0),
                        stop=(kk == 7),
                    )
                g2 = scr.tile([P, 512], BF16, tag="g2")
                nc.scalar.activation(
                    g2[:],
                    ps2[:],
                    GELU,
                    bias=b2sb[:, m2 : m2 + 1],
                    scale=1.0,
                    accum_out=h2sl[:, m2 * 8 + c : m2 * 8 + c + 1],
                )

        # ---- finalize -------------------------------------------------
        if DO_TP:
            for th in range(4):
                for b in range(B):
                    nc.vector.tensor_reduce(
                        sums_sb[:, th * 4 + b : th * 4 + b + 1],
                        qsums[:, th * 32 + b * 8 : th * 32 + (b + 1) * 8],
                        AX,
                        ADD,
                    )
            nc.sync.dma_start(out=sums[:], in_=sums_sb[:])
        if DO_CUBIC:
            nc.vector.tensor_reduce(cub_sb[:, 0:1], pppsl[:], AX, ADD)
            nc.vector.tensor_reduce(cub_sb[:, 1:2], qpqsl[:], AX, ADD)
            nc.sync.dma_start(out=cub[:], in_=cub_sb[:])
        if DO_MLP:
            for m2 in range(4):
                nc.vector.tensor_reduce(
                    h2sum[:, m2 : m2 + 1], h2sl[:, m2 * 8 : (m2 + 1) * 8], AX, ADD
                )
            d3 = scr.tile([P, 4], F32, tag="mlpd")
            nc.vector.tensor_mul(d3[:], h2sum[:], w3sb[:])
            nc.vector.tensor_reduce(mlp_sb[:], d3[:], AX, ADD)
            nc.sync.dma_start(out=mlp[:], in_=mlp_sb[:])

    nc.compile()
    return nc


def _get_nc():
    if "nc" not in _CACHED:
        _CACHED["nc"] = _build_program()
    return _CACHED["nc"]


def _prep_in_maps(q, p, coef_args):
    (qqq_w1, qqq_w2, qqq_w3, ppp_w1, ppp_w2, ppp_w3,
     mlp_w1, mlp_b1, mlp_w2, mlp_b2, mlp_w3, mlp_b3) = coef_args
    bf = ml_dtypes.bfloat16
    wp_np = np.concatenate(
        [np.asarray(ppp_w1), np.asarray(ppp_w2), np.asarray(ppp_w3)], axis=1
    ).astype(bf)
    wq_np = np.asarray(qqq_w3).astype(bf)
    w1_np = np.asarray(mlp_w1).astype(bf)
    w2_np = np.asarray(mlp_w2).astype(bf)
    b1_np = np.asarray(mlp_b1, dtype=np.float32).reshape(8, P).T.copy()
    b2_np = np.asarray(mlp_b2, dtype=np.float32).reshape(4, P).T.copy()
    w3_np = np.asarray(mlp_w3, dtype=np.float32).reshape(4, P).T.copy()

    qf = np.ascontiguousarray(np.asarray(q, dtype=np.float32))
    pf = np.ascontiguousarray(np.asarray(p, dtype=np.float32))

    in_maps = []
    for c in range(NCORES):
        qs_c = np.ascontiguousarray(
            qf[:, c * SS : (c + 1) * SS, :]
        ).reshape(T, D)
        ps_c = np.ascontiguousarray(
            pf[:, c * SS : (c + 1) * SS, :]
        ).reshape(T, D)
        in_maps.append(
            {
                "qs": qs_c,
                "ps": ps_c,
                "wp": wp_np,
                "wq": wq_np,
                "w1": w1_np,
                "w2": w2_np,
                "b1c": b1_np,
                "b2c": b2_np,
                "w3c": w3_np,
            }
        )
    return in_maps


def _finalize(results, q, p, coef_linear_q, coef_linear_p,
              coef_quadratic_qp, coef_quadratic_qq, coef_quadratic_pp,
              h_offset, mlp_b3):
    q_sum = np.zeros((B, D), dtype=np.float64)
    p_sum = np.zeros((B, D), dtype=np.float64)
    ppp = 0.0
    qpq = 0.0
    mlp_t = 0.0
    for r in results:
        sums = np.asarray(r["sums"], dtype=np.float64)   # [128, 16]
        for t in range(2):
            tgt = q_sum if t == 0 else p_sum
            for h in range(2):
                for b in range(B):
                    col = (t * 2 + h) * 4 + b
                    tgt[b, h * P : (h + 1) * P] += sums[:, col]
        cubv = np.asarray(r["cub"], dtype=np.float64)
        ppp += cubv[:, 0].sum()
        qpq += cubv[:, 1].sum()
        mlp_t += np.asarray(r["mlp"], dtype=np.float64)[:, 0].sum()

    c_lq = np.asarray(coef_linear_q, dtype=np.float64)
    c_lp = np.asarray(coef_linear_p, dtype=np.float64)
    lin_q = float((q_sum @ c_lq).sum())
    lin_p = float((p_sum @ c_lp).sum())

    def quad(cmat, a_sum, b_sum):
        csum = np.asarray(cmat, dtype=np.float64).sum(axis=1)
        return float(np.einsum("bd,d,bd->", a_sum, csum, b_sum))

    quad_qp = quad(coef_quadratic_qp, q_sum, p_sum)
    quad_qq = quad(coef_quadratic_qq, q_sum, q_sum)
    quad_pp = quad(coef_quadratic_pp, p_sum, p_sum)

    cubic = 3.0 * ppp + qpq
    mlp_total = mlp_t + B * S * float(np.asarray(mlp_b3).reshape(-1)[0])

    H = (
        B * float(np.asarray(h_offset).reshape(-1)[0])
        + lin_q + lin_p + quad_qp + quad_qq + quad_pp + cubic + mlp_total
    )
    return np.float32(H)


def kernel(q, p, coef_linear_q, coef_linear_p,
           coef_quadratic_qp, coef_quadratic_qq, coef_quadratic_pp,
           h_offset, qqq_w1, qqq_w2, qqq_w3, ppp_w1, ppp_w2, ppp_w3,
           mlp_w1, mlp_b1, mlp_w2, mlp_b2, mlp_w3, mlp_b3):
    nc = _get_nc()
    in_maps = _prep_in_maps(
        q, p,
        (qqq_w1, qqq_w2, qqq_w3, ppp_w1, ppp_w2, ppp_w3,
         mlp_w1, mlp_b1, mlp_w2, mlp_b2, mlp_w3, mlp_b3),
    )
    res = run_bass_kernel_spmd(nc, in_maps, list(range(NCORES))).results
    H = _finalize(
        res, q, p, coef_linear_q, coef_linear_p,
        coef_quadratic_qp, coef_quadratic_qq, coef_quadratic_pp,
        h_offset, mlp_b3,
    )
    qf = np.asarray(q, dtype=np.float32)
    pf = np.asarray(p, dtype=np.float32)
    return (qf, pf, H)


# revision 20
# speedup vs baseline: 1.0479x; 1.0479x over previous
"""Trainium2 Bass kernel for nn_HamiltonianBlock.

Reference computes (q, p, H) where q,p pass through unchanged and H is a
scalar: sums of linear/quadratic terms of per-batch token sums, low-rank
cubic terms, and a token MLP, all reduced over batch+sequence.

Strategy (8 cores, data-parallel over sequence):
  - shard tokens: core c gets q[:, c*1024:(c+1)*1024, :] -> [4096, 256]
  - on device per core:
      * PE transposes fp32 token tiles -> xT (feature-major), DVE
        tensor_tensor_reduce evacuates PSUM -> bf16 xT while emitting
        exact fp32 per-feature token sums (q_sum / p_sum partials)
      * cubic projections p1,p2,p3,q3 via bf16 matmuls (token-major out),
        DVE elementwise + reduce for ppp / qpq partials
      * token MLP in bf16 on PE; gelu (erf) + bias on ACT; final layer
        folded into a per-feature token sum (accum_out) dotted with w3
  - host: gather tiny per-core partials, finish the O(B*D) algebra in
    float64, return (q, p, H).

Precision: H is dominated (~99.9%) by the quadratic terms, which are
computed from exact fp32 token sums; the bf16 MLP/cubic parts contribute
~1.5e-3 of |H| so their bf16 error is ~1e-6 relative on H.
"""

import os

os.environ.setdefault("BASS_NEVER_TRACE", "1")

import numpy as np
import ml_dtypes

from contextlib import ExitStack

import concourse.bass as bass
import concourse.tile as tile
from concourse import bacc, mybir
from concourse.bass_utils import run_bass_kernel_spmd
from concourse.masks import make_identity

F32 = mybir.dt.float32
BF16 = mybir.dt.bfloat16

B, S, D = 4, 8192, 256
HID = 512
R = 32
NCORES = 8
SS = S // NCORES            # 1024 tokens of each batch per core
T = B * SS                  # 4096 tokens per core
P = 128                     # partition dim
NCHUNK = T // 512           # 8 chunks of 512 tokens
NBLK = 4                    # 128-token blocks per chunk

_CACHED = {}

DO_CUBIC = os.environ.get("K_NO_CUBIC", "") == ""
DO_MLP = os.environ.get("K_NO_MLP", "") == ""
DO_TP = os.environ.get("K_NO_TP", "") == ""
USE_FP32R_TP = os.environ.get("K_FP32R", "") != ""


def _build_program():
    nc = bacc.Bacc(
        "TRN2",
        target_bir_lowering=False,
        debug=False,
        num_devices=NCORES,
    )

    qs = nc.dram_tensor("qs", [T, D], F32, kind="ExternalInput").ap()
    ps = nc.dram_tensor("ps", [T, D], F32, kind="ExternalInput").ap()
    wp = nc.dram_tensor("wp", [D, 3 * R], BF16, kind="ExternalInput").ap()
    wq = nc.dram_tensor("wq", [D, R], BF16, kind="ExternalInput").ap()
    w1 = nc.dram_tensor("w1", [2 * D, 2 * HID], BF16, kind="ExternalInput").ap()
    w2 = nc.dram_tensor("w2", [2 * HID, HID], BF16, kind="ExternalInput").ap()
    b1c = nc.dram_tensor("b1c", [P, 8], F32, kind="ExternalInput").ap()
    b2c = nc.dram_tensor("b2c", [P, 4], F32, kind="ExternalInput").ap()
    w3c = nc.dram_tensor("w3c", [P, 4], F32, kind="ExternalInput").ap()

    sums = nc.dram_tensor("sums", [P, 16], F32, kind="ExternalOutput").ap()
    cub = nc.dram_tensor("cub", [P, 2], F32, kind="ExternalOutput").ap()
    mlp = nc.dram_tensor("mlp", [P, 1], F32, kind="ExternalOutput").ap()

    MULT = mybir.AluOpType.mult
    ADD = mybir.AluOpType.add
    AX = mybir.AxisListType.X
    GELU = mybir.ActivationFunctionType.Gelu

    with tile.TileContext(nc) as tc, ExitStack() as ctx:
        consts = ctx.enter_context(tc.tile_pool(name="consts", bufs=1))
        wpool = ctx.enter_context(tc.tile_pool(name="wpool", bufs=1))
        accp = ctx.enter_context(tc.tile_pool(name="accp", bufs=1))
        xtp_pool = ctx.enter_context(tc.tile_pool(name="xt", bufs=1))
        xin = ctx.enter_context(tc.tile_pool(name="xin", bufs=3))
        scr = ctx.enter_context(tc.tile_pool(name="scr", bufs=3))

        if DO_MLP:
            h1p = ctx.enter_context(tc.tile_pool(name="h1p", bufs=12))
            mm1_ps = ctx.enter_context(
                tc.tile_pool(name="mm1_ps", bufs=2, space=bass.MemorySpace.PSUM)
            )
            mm2_ps = ctx.enter_context(
                tc.tile_pool(name="mm2_ps", bufs=2, space=bass.MemorySpace.PSUM)
            )
        if DO_TP:
            tp_ps = ctx.enter_context(
                tc.tile_pool(name="tp_ps", bufs=3, space=bass.MemorySpace.PSUM)
            )
        if DO_CUBIC:
            cub_ps = ctx.enter_context(
                tc.tile_pool(name="cub_ps", bufs=1, space=bass.MemorySpace.PSUM)
            )

        # ---- constants & weights -------------------------------------
        identity = consts.tile([P, P], F32)
        make_identity(nc, identity[:])

        w1sb = wpool.tile([P, 4 * 2 * HID], BF16)   # [128, 4096]
        for k in range(4):
            nc.sync.dma_start(
                out=w1sb[:, k * 1024 : (k + 1) * 1024],
                in_=w1[k * P : (k + 1) * P, :],
            )
        w2sb = wpool.tile([P, 8 * HID], BF16)       # [128, 4096]
        for k in range(8):
            nc.sync.dma_start(
                out=w2sb[:, k * HID : (k + 1) * HID],
                in_=w2[k * P : (k + 1) * P, :],
            )
        wpsb = wpool.tile([P, 2 * 3 * R], BF16)     # [128, 192]
        wqsb = wpool.tile([P, 2 * R], BF16)         # [128, 64]
        for h in range(2):
            nc.sync.dma_start(
                out=wpsb[:, h * 96 : (h + 1) * 96], in_=wp[h * P : (h + 1) * P, :]
            )
            nc.sync.dma_start(
                out=wqsb[:, h * R : (h + 1) * R], in_=wq[h * P : (h + 1) * P, :]
            )
        b1sb = wpool.tile([P, 8], F32)
        nc.sync.dma_start(out=b1sb[:], in_=b1c[:])
        b2sb = wpool.tile([P, 4], F32)
        nc.sync.dma_start(out=b2sb[:], in_=b2c[:])
        w3sb = wpool.tile([P, 4], F32)
        nc.sync.dma_start(out=w3sb[:], in_=w3c[:])

        # ---- accumulators --------------------------------------------
        if DO_TP:
            qsums = accp.tile([P, 4 * 32], F32)  # (t*2+half)*32 + chunk*4+blk
            sums_sb = accp.tile([P, 16], F32)
        if DO_CUBIC:
            pppsl = accp.tile([P, 32], F32)
            qpqsl = accp.tile([P, 32], F32)
            cub_sb = accp.tile([P, 2], F32)
        if DO_MLP:
            h2sl = accp.tile([P, 32], F32)       # m2*8 + chunk
            h2sum = accp.tile([P, 4], F32)
            mlp_sb = accp.tile([P, 1], F32)

        # xT buffers: [128, half, T] feature-major bf16
        xtq = xtp_pool.tile([P, 2, T], BF16)
        xtp = xtp_pool.tile([P, 2, T], BF16)

        for c in range(NCHUNK):
            tok0 = c * 512
            qin = xin.tile([P, NBLK, D], F32, tag="qin")
            nc.sync.dma_start(
                out=qin[:],
                in_=qs[tok0 : tok0 + 512, :].rearrange("(n p) d -> p n d", p=P),
            )
            pin = xin.tile([P, NBLK, D], F32, tag="pin")
            nc.sync.dma_start(
                out=pin[:],
                in_=ps[tok0 : tok0 + 512, :].rearrange("(n p) d -> p n d", p=P),
            )

            # transposes + token-sum partials + bf16 evacuation
            for t, (xi, xt) in enumerate(((qin, xtq), (pin, xtp))) if DO_TP else ():
                for h in range(2):
                    for j in range(NBLK):
                        tin = xi[:, j, h * P : (h + 1) * P]
                        tid = identity[:]
                        if USE_FP32R_TP:
                            pst = tp_ps.tile([P, P], mybir.dt.float32r)
                            nc.tensor.transpose(
                                pst[:],
                                tin.bitcast(mybir.dt.float32r),
                                tid.bitcast(mybir.dt.float32r),
                            )
                            psb = pst[:].bitcast(F32)
                        else:
                            pst = tp_ps.tile([P, P], F32)
                            nc.tensor.transpose(pst[:], tin, tid)
                            psb = pst[:]
                        slot = (t * 2 + h) * 32 + c * 4 + j
                        nc.vector.tensor_copy(
                            xt[:, h, tok0 + j * P : tok0 + (j + 1) * P], psb
                        )
                        nc.vector.tensor_reduce(
                            qsums[:, slot : slot + 1], psb, AX, ADD
                        )

            # cubic terms, token-major [128 tok, R]
            for j in range(NBLK) if DO_CUBIC else ():
                cps = cub_ps.tile([P, P], F32)
                for h in range(2):
                    nc.tensor.matmul(
                        cps[:, 0:96],
                        lhsT=xtp[:, h, tok0 + j * P : tok0 + (j + 1) * P],
                        rhs=wpsb[:, h * 96 : (h + 1) * 96],
                        start=(h == 0),
                        stop=(h == 1),
                        skip_group_check=True,
                    )
                for h in range(2):
                    nc.tensor.matmul(
                        cps[:, 96:128],
                        lhsT=xtq[:, h, tok0 + j * P : tok0 + (j + 1) * P],
                        rhs=wqsb[:, h * R : (h + 1) * R],
                        start=(h == 0),
                        stop=(h == 1),
                        skip_group_check=True,
                    )
                s12 = scr.tile([P, 2 * R], F32, tag="s12")
                nc.vector.tensor_copy(s12[:], cps[:, 0:64])
                p12 = scr.tile([P, R], F32, tag="p12")
                nc.vector.tensor_mul(p12[:], s12[:, 0:32], s12[:, 32:64])
                d1 = scr.tile([P, R], F32, tag="ttrd")
                nc.vector.tensor_mul(d1[:], p12[:], cps[:, 64:96])
                nc.vector.tensor_reduce(
                    pppsl[:, c * 4 + j : c * 4 + j + 1], d1[:], AX, ADD
                )
                d2 = scr.tile([P, R], F32, tag="ttrd")
                nc.vector.tensor_mul(d2[:], p12[:], cps[:, 96:128])
                nc.vector.tensor_reduce(
                    qpqsl[:, c * 4 + j : c * 4 + j + 1], d2[:], AX, ADD
                )

            # MLP layer 1: h1T[m] = gelu(w1[:,m].T @ xT + b1[m])
            h1list = []
            for m in range(8) if DO_MLP else ():
                ps1 = mm1_ps.tile([P, 512], F32)
                for k in range(4):
                    xt = xtq if k < 2 else xtp
                    h = k % 2
                    nc.tensor.matmul(
                        ps1[:],
                        lhsT=w1sb[:, k * 1024 + m * P : k * 1024 + (m + 1) * P],
                        rhs=xt[:, h, tok0 : tok0 + 512],
                        start=(k == 0),
                        stop=(k == 3),
                    )
                h1t = h1p.tile([P, 512], BF16)
                nc.scalar.activation(
                    h1t[:], ps1[:], GELU, bias=b1sb[:, m : m + 1], scale=1.0
                )
                h1list.append(h1t)

            # MLP layer 2 + fused token reduction
            for m2 in range(4) if DO_MLP else ():
                ps2 = mm2_ps.tile([P, 512], F32)
                for kk in range(8):
                    nc.tensor.matmul(
                        ps2[:],
                        lhsT=w2sb[:, kk * HID + m2 * P : kk * HID + (m2 + 1) * P],
                        rhs=h1list[kk][:],
                        start=(kk == 0),
                        stop=(kk == 7),
                    )
                g2 = scr.tile([P, 512], BF16, tag="g2")
                nc.scalar.activation(
                    g2[:],
                    ps2[:],
                    GELU,
                    bias=b2sb[:, m2 : m2 + 1],
                    scale=1.0,
                    accum_out=h2sl[:, m2 * 8 + c : m2 * 8 + c + 1],
                )

        # ---- finalize -------------------------------------------------
        if DO_TP:
            for th in range(4):
                for b in range(B):
                    nc.vector.tensor_reduce(
                        sums_sb[:, th * 4 + b : th * 4 + b + 1],
                        qsums[:, th * 32 + b * 8 : th * 32 + (b + 1) * 8],
                        AX,
                        ADD,
                    )
            nc.sync.dma_start(out=sums[:], in_=sums_sb[:])
        if DO_CUBIC:
            nc.vector.tensor_reduce(cub_sb[:, 0:1], pppsl[:], AX, ADD)
            nc.vector.tensor_reduce(cub_sb[:, 1:2], qpqsl[:], AX, ADD)
            nc.sync.dma_start(out=cub[:], in_=cub_sb[:])
        if DO_MLP:
            for m2 in range(4):
                nc.vector.tensor_reduce(
                    h2sum[:, m2 : m2 + 1], h2sl[:, m2 * 8 : (m2 + 1) * 8], AX, ADD
                )
            d3 = scr.tile([P, 4], F32, tag="mlpd")
            nc.vector.tensor_mul(d3[:], h2sum[:], w3sb[:])
            nc.vector.tensor_reduce(mlp_sb[:], d3[:], AX, ADD)
            nc.sync.dma_start(out=mlp[:], in_=mlp_sb[:])

    nc.compile()
    return nc


def _get_nc():
    if "nc" not in _CACHED:
        _CACHED["nc"] = _build_program()
    return _CACHED["nc"]


def _prep_in_maps(q, p, coef_args):
    (qqq_w1, qqq_w2, qqq_w3, ppp_w1, ppp_w2, ppp_w3,
     mlp_w1, mlp_b1, mlp_w2, mlp_b2, mlp_w3, mlp_b3) = coef_args
    bf = ml_dtypes.bfloat16
    wp_np = np.concatenate(
        [np.asarray(ppp_w1), np.asarray(ppp_w2), np.asarray(ppp_w3)], axis=1
    ).astype(bf)
    wq_np = np.asarray(qqq_w3).astype(bf)
    w1_np = np.asarray(mlp_w1).astype(bf)
    w2_np = np.asarray(mlp_w2).astype(bf)
    b1_np = np.asarray(mlp_b1, dtype=np.float32).reshape(8, P).T.copy()
    b2_np = np.asarray(mlp_b2, dtype=np.float32).reshape(4, P).T.copy()
    w3_np = np.asarray(mlp_w3, dtype=np.float32).reshape(4, P).T.copy()

    qf = np.ascontiguousarray(np.asarray(q, dtype=np.float32))
    pf = np.ascontiguousarray(np.asarray(p, dtype=np.float32))

    in_maps = []
    for c in range(NCORES):
        qs_c = np.ascontiguousarray(
            qf[:, c * SS : (c + 1) * SS, :]
        ).reshape(T, D)
        ps_c = np.ascontiguousarray(
            pf[:, c * SS : (c + 1) * SS, :]
        ).reshape(T, D)
        in_maps.append(
            {
                "qs": qs_c,
                "ps": ps_c,
                "wp": wp_np,
                "wq": wq_np,
                "w1": w1_np,
                "w2": w2_np,
                "b1c": b1_np,
                "b2c": b2_np,
                "w3c": w3_np,
            }
        )
    return in_maps


def _finalize(results, q, p, coef_linear_q, coef_linear_p,
              coef_quadratic_qp, coef_quadratic_qq, coef_quadratic_pp,
              h_offset, mlp_b3):
    q_sum = np.zeros((B, D), dtype=np.float64)
    p_sum = np.zeros((B, D), dtype=np.float64)
    ppp = 0.0
    qpq = 0.0
    mlp_t = 0.0
    for r in results:
        sums = np.asarray(r["sums"], dtype=np.float64)   # [128, 16]
        for t in range(2):
            tgt = q_sum if t == 0 else p_sum
            for h in range(2):
                for b in range(B):
                    col = (t * 2 + h) * 4 + b
                    tgt[b, h * P : (h + 1) * P] += sums[:, col]
        cubv = np.asarray(r["cub"], dtype=np.float64)
        ppp += cubv[:, 0].sum()
        qpq += cubv[:, 1].sum()
        mlp_t += np.asarray(r["mlp"], dtype=np.float64)[:, 0].sum()

    c_lq = np.asarray(coef_linear_q, dtype=np.float64)
    c_lp = np.asarray(coef_linear_p, dtype=np.float64)
    lin_q = float((q_sum @ c_lq).sum())
    lin_p = float((p_sum @ c_lp).sum())

    def quad(cmat, a_sum, b_sum):
        csum = np.asarray(cmat, dtype=np.float64).sum(axis=1)
        return float(np.einsum("bd,d,bd->", a_sum, csum, b_sum))

    quad_qp = quad(coef_quadratic_qp, q_sum, p_sum)
    quad_qq = quad(coef_quadratic_qq, q_sum, q_sum)
    quad_pp = quad(coef_quadratic_pp, p_sum, p_sum)

    cubic = 3.0 * ppp + qpq
    mlp_total = mlp_t + B * S * float(np.asarray(mlp_b3).reshape(-1)[0])

    H = (
        B * float(np.asarray(h_offset).reshape(-1)[0])
        + lin_q + lin_p + quad_qp + quad_qq + quad_pp + cubic + mlp_total
    )
    return np.float32(H)


def kernel(q, p, coef_linear_q, coef_linear_p,
           coef_quadratic_qp, coef_quadratic_qq, coef_quadratic_pp,
           h_offset, qqq_w1, qqq_w2, qqq_w3, ppp_w1, ppp_w2, ppp_w3,
           mlp_w1, mlp_b1, mlp_w2, mlp_b2, mlp_w3, mlp_b3):
    nc = _get_nc()
    in_maps = _prep_in_maps(
        q, p,
        (qqq_w1, qqq_w2, qqq_w3, ppp_w1, ppp_w2, ppp_w3,
         mlp_w1, mlp_b1, mlp_w2, mlp_b2, mlp_w3, mlp_b3),
    )
    res = run_bass_kernel_spmd(nc, in_maps, list(range(NCORES))).results
    H = _finalize(
        res, q, p, coef_linear_q, coef_linear_p,
        coef_quadratic_qp, coef_quadratic_qq, coef_quadratic_pp,
        h_offset, mlp_b3,
    )
    qf = np.asarray(q, dtype=np.float32)
    pf = np.asarray(p, dtype=np.float32)
    return (qf, pf, H)
